# revision 33
# baseline (speedup 1.0000x reference)
"""Trainium2 Bass kernel for nn_CGAMotorModel.

Reference computes, for B=512, H=1024, D=5 multivector channels of Cl(4,1):
    W_x[b,h]  = sum_d x[b,d] o W_in[h,d]          (o = geometric product)
    h_free    = (1 - (1-dt)^n) * W_x              (closed form of the scan)
    out[b]    = sum_h h_free[b,h] o W_out[h]

By bilinearity this collapses to out[b] = c * sum_d x[b,d] o K_d with
K_d = sum_h W_in[h,d] o W_out[h] and c = 1 - 0.9^10.  H-tensor-parallel:
each core takes a 128-row H chunk, builds its partial M[(p,d), m] via
    S^T[r,(d,q)] = sum_h W_out[h,r] W_in[h,(d,q)]   (2 matmuls, K=128)
    K^T[r',d]    = sum_q (c*C[q]).T @ S_q^T         (8 matmuls, K=128)
    M^T[m,(p,d)] = per-p C[p] slab @ K^T            (32 matmuls, N=5)
    M            = PE-transpose of M^T              (2 transposes)
    out[b,m]     = X^T.T @ M                        (8 matmuls)
and the host sums the 8 partial outputs.  x arrives host-pretransposed
(X^T[(p,d), b], p-major) so no on-device transposes of x are needed.

Cost-model-driven choices, verified on silicon (walrus BIR verifier +
burst runs; CoreSim alone is insufficient):
 - bf16 operands everywhere (1 PE cycle/row vs 4 for fp32; rel tol 2e-2)
 - GPSIMD cannot touch PSUM -> DVE+ACT do all PSUM evacuation, with
   producers split into per-consumer PSUM tiles (same-tile readers get
   serialized by the tile dep tracker)
 - ALL matmul operands at partition base 0: base-32/64 operands raced
   intermittently on real silicon, so both Cayley halves are packed as
   [32, 1024] base-0 slab rows
 - ACT issues no DMAs so its 1283ns activation-table load schedules at
   t~200 (forced by an early warm-up copy), off the critical path
 - one DMA per logical input (SP: w, cc2; Pool: cc1, xt1, xt2), single
   64KB store [128, (t m)]; host de-interleaves and sums partial outputs
"""

import numpy as np
import ml_dtypes

import concourse.bass as bass
import concourse.mybir as mybir
import concourse.tile as tile
from concourse import bacc
from concourse.bass_utils import run_bass_kernel_spmd
from concourse.masks import make_identity

B, H, D, MV = 512, 1024, 5, 32
N_CORES = 8
H_LOC = H // N_CORES
DT, N_FREE = 0.1, 10
C_SCALE = 1.0 - (1.0 - DT) ** N_FREE
F32 = mybir.dt.float32
BF16 = mybir.dt.bfloat16
NP_BF16 = np.dtype(ml_dtypes.bfloat16)


def _cayley_np() -> np.ndarray:
    """Cayley table for Cl(4,1), metric diag(1,1,1,1,-1). C[a,b,a^b] = sign."""
    metric = np.array([1.0, 1.0, 1.0, 1.0, -1.0], dtype=np.float32)
    C = np.zeros((32, 32, 32), dtype=np.float32)
    for a in range(32):
        for b in range(32):
            cnt = 0
            aa = a >> 1
            while aa:
                cnt += bin(aa & b).count("1")
                aa >>= 1
            s = -1.0 if (cnt & 1) else 1.0
            common = a & b
            for i in range(5):
                if (common >> i) & 1:
                    s *= metric[i]
            C[a, b, a ^ b] = s
    return C


def _pack_cayley():
    """cc1[128, 256]: cc1[32*(q//8)+r, 32*(q%8)+r'] = C_SCALE * C[q, r, r']
    (K-step lhsT, contraction over all 128 partitions = 4 q's per matmul).
    cc2[32, 1024]: cc2[r', 32*p+m] = C[p, r', m]  (Mt-step lhsT, base 0)."""
    C = _cayley_np()
    cc1 = np.concatenate(
        [C_SCALE * C[q] for q in range(32)], axis=1
    )  # [32, 1024]
    cc2 = np.concatenate([C[p] for p in range(32)], axis=1)  # [32, 1024]
    return (
        np.ascontiguousarray(cc1).astype(NP_BF16),
        np.ascontiguousarray(cc2).astype(NP_BF16),
    )


CC1, CC2 = _pack_cayley()


def build_program() -> bass.Bass:
    nc = bacc.Bacc()
    # wT = (per-core [W_in.reshape(H,160) | W_out.reshape(H,32)] chunk).T
    wT = nc.dram_tensor("wT", [H_LOC, 192], BF16, kind="ExternalInput")
    cc1 = nc.dram_tensor("cc1", [32, 1024], BF16, kind="ExternalInput")
    cc2 = nc.dram_tensor("cc2", [32, 1024], BF16, kind="ExternalInput")
    # xt1/xt2 = X^T[(p,d), b] rows 0:128 / 128:160
    xt1 = nc.dram_tensor("xt1", [128, B], BF16, kind="ExternalInput")
    xt2 = nc.dram_tensor("xt2", [32, B], BF16, kind="ExternalInput")
    # out layout [p, (t m)]; host de-interleaves to [t*128+p, m] and sums cores
    out = nc.dram_tensor("out", [128, 4 * MV], F32, kind="ExternalOutput")

    with tile.TileContext(nc) as tc:
        with (
            tc.tile_pool(name="sb", bufs=1) as sb,
            tc.tile_pool(name="psA", bufs=1, space="PSUM") as psA,
            tc.tile_pool(name="psB", bufs=1, space="PSUM") as psB,
            tc.tile_pool(name="psO", bufs=1, space="PSUM") as psO,
        ):
            # --- loads.  GPSIMD cannot touch PSUM on real HW, so DVE+ACT do
            # all PSUM evacuation; ACT issues NO DMAs (its activation table
            # load then schedules at t~200, off the critical path).  All
            # matmul operands sit at partition base 0: base-32/64 operands
            # raced intermittently on real silicon.  cc2's 790ns transfer
            # hides behind the S->K->ksb chain.
            w_sb = sb.tile([128, 192], BF16, tag="w_sb")
            nc.sync.dma_start(w_sb[:], wT[:])
            cc2_sb = sb.tile([32, 1024], BF16, tag="cc2_sb")
            nc.sync.dma_start(cc2_sb[:], cc2[:])
            cc_sb = sb.tile([32, 1024], BF16, tag="cc_sb")
            nc.gpsimd.dma_start(cc_sb[:], cc1[:])
            xt1_sb = sb.tile([128, B], BF16, tag="xt1_sb")
            nc.gpsimd.dma_start(xt1_sb[:], xt1[:])
            xt2_sb = sb.tile([32, B], BF16, tag="xt2_sb")
            nc.gpsimd.dma_start(xt2_sb[:], xt2[:])
            ident = sb.tile([32, 32], BF16, tag="ident")
            make_identity(nc, ident[:])
            # warm-up Activation copy (activation-table hoist)
            warm = sb.tile([1, 1], F32, tag="warm")
            nc.scalar.copy(warm[:], nc.const_aps.aps[(F32, 0.0)][0:1, :])

            # --- S-step: S^T[r,(d,q)] = sum_h W_out[h,r] W_in[h,(d,q)] ---
            # Two PSUM tiles (q<16 / q>=16) so DVE and ACT evacuate in
            # parallel (same-tile PSUM readers get serialized by the dep
            # tracker).  spkB also hosts K^T at cols 80:85 (bank budget).
            spsumA = psA.tile([32, 80], F32, tag="spsumA")
            spkB = psA.tile([32, 88], F32, tag="spkB")
            wv = w_sb[:, 0:160].rearrange("h (d q) -> h d q", d=D)
            nc.tensor.matmul(
                spsumA[:].rearrange("r (d q) -> r d q", d=D),
                w_sb[:, 160:192],
                wv[:, :, 0:16],
                start=True,
                stop=True,
            )
            nc.tensor.matmul(
                spkB[:, 0:80].rearrange("r (d q) -> r d q", d=D),
                w_sb[:, 160:192],
                wv[:, :, 16:32],
                start=True,
                stop=True,
            )
            # evacuate S^T to SBUF in q-half-blocked layout [32, (h,d,q16)]
            # matching the PSUM tiles exactly, so both evac copies are pure
            # flat copies (a 3-D strided out-AP costs ACT ~40ns extra)
            ssb = sb.tile([32, 160], BF16, tag="ssb")
            nc.vector.tensor_copy(ssb[:, 0:80], spsumA[:])
            nc.scalar.copy(ssb[:, 80:160], spkB[:, 0:80])

            # --- K-step: K^T[r',d] = sum_q (c*C[q]).T @ S_q^T ---
            # rhs for q: cols {16d + (q%16)} within q-half block
            kpsum = spkB[:, 80:85]
            for q in range(32):
                base = 80 * (q // 16) + (q % 16)
                nc.tensor.matmul(
                    kpsum,
                    cc_sb[:, 32 * q : 32 * q + 32],
                    ssb[:, base : base + 65 : 16],
                    start=(q == 0),
                    stop=(q == 31),
                )
            ksb = sb.tile([32, D], BF16, tag="ksb")
            nc.vector.tensor_copy(ksb[:], kpsum)

            # --- Mt-step: M^T[m, 5p+d] = sum_r' C[p,r',m] K^T[r',d] ---
            mtpA = psB.tile([32, 110], F32, tag="mtpA")
            mtpB = psB.tile([32, 50], F32, tag="mtpB")
            for p in range(32):
                dst = mtpA[:, 5 * p : 5 * p + 5] if p < 22 else mtpB[
                    :, 5 * (p - 22) : 5 * (p - 22) + 5
                ]
                nc.tensor.matmul(
                    dst,
                    cc2_sb[:, 32 * p : 32 * p + 32],
                    ksb[:],
                    start=True,
                    stop=True,
                )
            mtsb = sb.tile([32, 160], BF16, tag="mtsb")
            nc.vector.tensor_copy(mtsb[:, 0:110], mtpA[:])
            nc.scalar.copy(mtsb[:, 110:160], mtpB[:])

            # --- M = M^T.T via PE transposes (partition order = (p,d) flat,
            # matching the host-pretransposed x) ---
            m2p = psB.tile([32, MV], BF16, tag="m2p")
            nc.tensor.transpose(m2p[:], mtsb[:, 128:160], ident[:])
            m1p = psB.tile([128, MV], BF16, tag="m1p")
            nc.tensor.transpose(m1p[:], mtsb[:, 0:128], ident[:])
            m2 = sb.tile([32, MV], BF16, tag="m2")
            nc.scalar.copy(m2[:], m2p[:])
            m1 = sb.tile([128, MV], BF16, tag="m1")
            nc.vector.tensor_copy(m1[:], m1p[:])

            # --- final: out[b,m] = sum_(p,d) X^T[(p,d),b] M[(p,d),m] ---
            osb = sb.tile([128, 4 * MV], F32, tag="osb")
            opsA = psO.tile([128, 2 * MV], F32, tag="opsA")
            opsB = psO.tile([128, 2 * MV], F32, tag="opsB")
            for t in range(4):
                ops = opsA if t < 2 else opsB
                dst = ops[:, MV * (t % 2) : MV * (t % 2) + MV]
                nc.tensor.matmul(
                    dst,
                    xt1_sb[:, 128 * t : 128 * (t + 1)],
                    m1[:],
                    start=True,
                    stop=False,
                )
                nc.tensor.matmul(
                    dst,
                    xt2_sb[:, 128 * t : 128 * (t + 1)],
                    m2[:],
                    start=False,
                    stop=True,
                )
            nc.vector.tensor_copy(osb[:, 0 : 2 * MV], opsA[:])
            nc.scalar.copy(osb[:, 2 * MV : 4 * MV], opsB[:])
            nc.sync.dma_start(out[:], osb[:])

    nc.finalize()
    return nc


def host_inputs(x_mv: np.ndarray, W_in: np.ndarray, W_out: np.ndarray):
    """Marshal full inputs into per-core DRAM tensors (pure data movement +
    dtype cast; all arithmetic stays on device)."""
    x_mv = np.asarray(x_mv)
    W_in = np.asarray(W_in)
    W_out = np.asarray(W_out)

    wcat = np.concatenate(
        [
            W_in.reshape(H, D * MV).astype(np.float32),
            W_out.reshape(H, MV).astype(np.float32),
        ],
        axis=1,
    )  # (1024, 192)

    # x host-pretransposed to X^T[(p,d), b], p-major flat index p*5+d
    xp = x_mv.astype(np.float32).transpose(0, 2, 1).reshape(B, MV * D).T  # (160, B)
    xt1 = np.ascontiguousarray(xp[0:128]).astype(NP_BF16)
    xt2 = np.ascontiguousarray(xp[128:160]).astype(NP_BF16)

    in_maps = []
    for c in range(N_CORES):
        wT = np.ascontiguousarray(
            wcat[H_LOC * c : H_LOC * (c + 1)]
        ).astype(NP_BF16)  # (128, 192)
        in_maps.append(
            {"wT": wT, "cc1": CC1, "cc2": CC2, "xt1": xt1, "xt2": xt2}
        )
    return in_maps


def host_output(parts) -> np.ndarray:
    """parts: list of 8 per-core [128, 128] partial outputs -> (B, 1, MV)."""
    acc = np.zeros((128, 4 * MV), dtype=np.float32)
    for p in parts:
        acc += np.asarray(p, dtype=np.float32)
    out = acc.reshape(128, 4, MV).transpose(1, 0, 2).reshape(B, MV)
    return np.ascontiguousarray(out, dtype=np.float32).reshape(B, 1, MV)


_NC_CACHE: list = []


def kernel(x_mv: np.ndarray, W_in: np.ndarray, W_out: np.ndarray) -> np.ndarray:
    if not _NC_CACHE:
        _NC_CACHE.append(build_program())
    nc = _NC_CACHE[0]
    in_maps = host_inputs(x_mv, W_in, W_out)
    import time as _time

    res = None
    for attempt in range(6):
        try:
            res = run_bass_kernel_spmd(nc, in_maps, core_ids=list(range(N_CORES)))
            break
        except Exception:
            # transient axon/NRT transport hiccups are common; retry
            if attempt == 5:
                raise
            _time.sleep(1.0 + attempt)
    parts = [res.results[c]["out"] for c in range(N_CORES)]
    return host_output(parts)
